# revision 7
# baseline (speedup 1.0000x reference)
"""Trainium2 Bass kernel for nn_BertOutput (binary-quantized BERT output layer).

Computation (see reference):
    w_scale = mean(|W|, axis=1)                  # [H, 1]
    W_q     = w_scale * sign(W)                  # [H, I]
    x_q     = clip * sign(x / clip)              # [B, S, I]
    h       = x_q @ W_q.T + b                    # [B, S, H]
    out     = LayerNorm(h + input_tensor) * gamma + beta

Structural facts exploited:
  * The matmul operands are exactly +-1: representable exactly in fp8e4m3,
    and the K=4096 accumulation of +-1 terms is exact in fp32 PSUM.  The
    per-output-channel scale (|clip| * mean|W|) is applied after the matmul.
  * fp8 enables MatmulPerfMode.DoubleRow: one instruction contracts TWO
    128-deep k-subtiles (157 TF/s peak), halving tensor-engine time vs bf16.
  * Sign bits survive the fp32->bf16 cast done during the DMA load.  x signs
    are packed PAIRWISE into u16 words -- fp8 sign of x[t, c] in the low
    byte, fp8 sign of x[t, 2048 + c] in the high byte -- with 3 contiguous
    DVE bitwise ops.  One 2-byte transpose then moves BOTH fp8 k-planes at
    once, and the byte-interleaved result is exactly what LDWEIGHTS perf
    mode DoubleRowSwInterleave consumes natively.  SwInterleave loads the
    first element to the largest PE column (reversing token order), which is
    cancelled by assigning tokens to SBUF partitions in reverse order when
    the shard is prepared on the host (a pure row permutation).
  * W-prep is sharded over the 8 cores: each core reads only a 512-row
    k-slice of W^T (2.1 MB instead of the full 16.8 MB), signs it to fp8 and
    computes its partial |W| column sums; one AllGather (plus the partial
    scale rows appended to the same buffer) shares the fp8 signs with every
    core.  The collective runs on TOPSP/SDMA silicon and its 4 MB payload is
    ~8x smaller than re-reading W in fp32, so both the DMA roofline and the
    serial W-prep phase (sign + ones.T @ |W| matmuls) shrink ~8x.
  * The gathered signs are read back per source rank in the pair order
    (0,4),(1,5),(2,6),(3,7): DoubleRow block b consumes k-tiles {b, b+16}
    which live in ranks {b//4, b//4 + 4}, so the main matmuls start after
    the FIRST 1 MB pair lands instead of after the whole 4 MB readback
    (tile's range-level hazard tracking gates each matmul on exactly the
    two readback DMAs it needs).
  * The per-channel scale numerator sum_k |W[h,k]| is a cross-partition
    reduction in the W^T layout, computed as ones.T @ |W^T| on the PE over
    the local k-slice; the 8 partial rows ride in the AllGather and are
    summed with one tiny fp32 matmul against a ones vector.
  * DMA-xbar transposes lock ALL DMA engines for their whole duration (they
    cannot overlap the HBM loads), so the x tile transposes run on the PE
    array instead (is_transpose matmul; the packed u16 words are bf16
    normals, so a bf16 PE transpose is bit-exact, HW-verified).

Sharding: data-parallel over tokens -- 8192 tokens -> 1024 per core; each
core computes a full LayerNorm over hidden=1024.  The only cross-core
exchange is the 0.5 MB/rank fp8 W-sign AllGather.
"""

import sys

sys.path.insert(0, "/opt/trn_rl_repo")

import numpy as np

import concourse.bass as bass  # noqa: F401  (import side effects / registry)
import concourse.tile as tile
from concourse import bacc, bass_utils, mybir

F32 = mybir.dt.float32
BF16 = mybir.dt.bfloat16
FP8 = mybir.dt.float8e4
U16 = mybir.dt.uint16
U8 = mybir.dt.uint8

HIDDEN = 1024
INTER = 4096
TOKENS = 8192
N_CORES = 8
TPC = TOKENS // N_CORES          # tokens per core = 1024
M_TILES = TPC // 128             # 8 token tiles per core
K_TILES = INTER // 128           # 32 k-tiles of W^T globally
KSLICE = INTER // N_CORES        # 512 k rows of W^T per core
KTPC = KSLICE // 128             # 4 k-tiles prepared per core
NBLK = INTER // 256              # 16 double-k-blocks (DoubleRow: 256 k each)
HALF = INTER // 2                # 2048: pack pairs (k, k + HALF)
EPS = 1e-12

TRACE = False                    # set True from test harness to profile
TRACE_ALL_CORES = False

_cache: dict = {}


def _install_ntff_hook():
    """The agent image's antenv package lacks axon_hooks, which silently
    disables NTFF profiling under axon.  Recreate it and wire the ctypes
    hook from trn_agent_boot (profiling/tooling only; the compute path
    does not depend on this)."""
    import types

    import antenv
    if getattr(antenv, "axon_hooks", None) is not None:
        return
    from trn_agent_boot.trn_boot import _ntff_profile_via_ctypes
    mod = types.ModuleType("antenv.axon_hooks")
    mod._hook = _ntff_profile_via_ctypes("/opt/axon/libaxon_pjrt.so")
    mod.get_axon_ntff_profile_hook = lambda: mod._hook

    def _set(h):
        mod._hook = h
    mod.set_axon_ntff_profile_hook = _set
    sys.modules["antenv.axon_hooks"] = mod
    antenv.axon_hooks = mod


def _prepare_x(x_shard: np.ndarray) -> np.ndarray:
    """Sharding-time row permutation: within each 128-token tile, tokens are
    assigned to SBUF partitions in REVERSE order, cancelling SwInterleave's
    first-element-to-largest-column reversal so psum rows come out natural."""
    t = x_shard.reshape(M_TILES, 128, INTER)
    return np.ascontiguousarray(t[:, ::-1, :]).reshape(TPC, INTER)


def _emit_pack(nc, pool, src, dst, tag):
    """Pack sign bits of a bf16 [128, 4096] tile into u16 fp8-sign pairs.

    dst u16 [128, 2048]: word c = lo byte fp8sign(src[:, c]),
                                  hi byte fp8sign(src[:, HALF + c]).
    fp8e4m3 +-1.0 is 0x38 / 0xB8, so:
        lo = (bf16_bits >> 8) & 0x0080  OR'd with  0x0038-from-tsB's 0x3838
        hi = (bf16_bits & 0x8000) | 0x3800
    """
    AT = mybir.AluOpType
    a = src[:, 0:HALF].bitcast(U16)
    b = src[:, HALF:INTER].bitcast(U16)
    tsA = pool.tile([128, HALF], U16, tag=f"{tag}A")
    nc.vector.tensor_scalar(out=tsA[:], in0=a, scalar1=8, scalar2=0x0080,
                            op0=AT.logical_shift_right, op1=AT.bitwise_and)
    tsB = pool.tile([128, HALF], U16, tag=f"{tag}B")
    nc.vector.tensor_scalar(out=tsB[:], in0=b, scalar1=0x8000, scalar2=0x3838,
                            op0=AT.bitwise_and, op1=AT.bitwise_or)
    nc.vector.tensor_tensor(out=dst[:], in0=tsA[:], in1=tsB[:],
                            op=AT.bitwise_or)


def _emit_program(nc, x_ap, res_ap, wt_ap, y_ap, b_ap, g_ap, be_ap,
                  scale_mul: float, use_b: bool, use_gamma: bool,
                  use_beta: bool):
    """Emit the per-core Tile program given DRAM APs.

    wt_ap is this core's k-slice of W TRANSPOSED ([KSLICE, HIDDEN]) -- a
    host-side layout/sharding choice so the weight lands k-major and needs
    no on-device transpose.
    """
    AT = mybir.AluOpType
    AF = mybir.ActivationFunctionType
    DRSI = mybir.MatmulPerfMode.DoubleRowSwInterleave
    RG = [list(range(N_CORES))]
    from concourse.masks import make_identity

    with tile.TileContext(nc) as tc:
        with (
            tc.tile_pool(name="wt", bufs=1) as wt_pool,
            tc.tile_pool(name="wstage", bufs=4) as wstage_pool,
            tc.tile_pool(name="wpk", bufs=2) as wpk_pool,
            tc.tile_pool(name="const", bufs=1) as const_pool,
            tc.tile_pool(name="xio", bufs=3) as xio_pool,
            tc.tile_pool(name="xpk", bufs=2) as xpk_pool,
            tc.tile_pool(name="xt", bufs=3) as xt_pool,
            tc.tile_pool(name="res", bufs=8) as res_pool,
            tc.tile_pool(name="epi", bufs=2) as epi_pool,
            tc.tile_pool(name="stats", bufs=2) as stats_pool,
            tc.tile_pool(name="psum", bufs=2, space="PSUM") as psum_pool,
            tc.tile_pool(name="pst", bufs=2, space="PSUM") as pst_pool,
            tc.tile_pool(name="wsps", bufs=1, space="PSUM") as wsps_pool,
            tc.tile_pool(name="dram", bufs=1, space="DRAM") as dram_pool,
        ):
            epsT = const_pool.tile([128, 1], F32, tag="epsT")
            nc.vector.memset(epsT[:], float(EPS))
            ones1 = const_pool.tile([128, 1], BF16, tag="ones1")
            nc.vector.memset(ones1[:], 1.0)
            ones8 = const_pool.tile([8, 1], F32, tag="ones8")
            nc.vector.memset(ones8[:], 1.0)
            ident = const_pool.tile([128, 128], BF16, tag="ident")
            make_identity(nc, ident[:])

            # ------------- DMA dispatch, phase 1 (gpsimd SWDGE ring) -------------
            # The ring drains strictly in dispatch order, so it doubles as a
            # priority list: the W k-slice first (it gates the AllGather
            # trigger), then the first two x tiles to feed the pack/transpose
            # front while the collective runs.
            wlds = []
            for c in range(KTPC):
                wld = wstage_pool.tile([128, HIDDEN], BF16, tag="wld")
                nc.gpsimd.dma_start(wld[:], wt_ap[c * 128:(c + 1) * 128, :])
                wlds.append(wld)

            xins, inps = {}, {}

            def dispatch_x(m):
                xin = xio_pool.tile([128, INTER], BF16, tag="xin")
                nc.gpsimd.dma_start(xin[:], x_ap[m * 128:(m + 1) * 128, :])
                xins[m] = xin

            for m in range(2):
                dispatch_x(m)

            # ---------------- local W-slice preparation ----------------
            # wT8s fp8 [128, KTPC, 1024]: (p, c, h) = fp8 sign W^T[c*128+p, h]
            # for this core's k-slice; wsps [1, 1024] = partial sum_k |W|.
            wT8s = wt_pool.tile([128, KTPC, HIDDEN], FP8, tag="wT8s",
                                name="wT8s")
            wsps = wsps_pool.tile([1, HIDDEN], F32, tag="wsps", name="wsps")

            for c in range(KTPC):
                wld = wlds[c]
                if c % 2 == 1:
                    # sign on the DVE (bit trick + convert) to run
                    # concurrently with the scalar engine's signs
                    wsg = wpk_pool.tile([128, HIDDEN], BF16, tag="wsg")
                    nc.vector.tensor_scalar(
                        out=wsg[:].bitcast(U16), in0=wld[:].bitcast(U16),
                        scalar1=0x8000, scalar2=0x3F80,
                        op0=AT.bitwise_and, op1=AT.bitwise_or)
                    nc.vector.tensor_copy(wT8s[:, c, :], wsg[:])
                else:
                    nc.scalar.sign(wT8s[:, c, :], wld[:])
                # |w| via sign-bit mask, then ones.T @ |w| accumulates the
                # per-channel scale numerator on the PE
                wabs = wpk_pool.tile([128, HIDDEN], BF16, tag="wabs")
                nc.vector.tensor_scalar(
                    out=wabs[:].bitcast(U16), in0=wld[:].bitcast(U16),
                    scalar1=0x7FFF, scalar2=None, op0=AT.bitwise_and)
                for n in range(2):
                    nc.tensor.matmul(wsps[:, n * 512:(n + 1) * 512],
                                     lhsT=ones1[:],
                                     rhs=wabs[:, n * 512:(n + 1) * 512],
                                     start=(c == 0),
                                     stop=(c == KTPC - 1))
            srow_part = const_pool.tile([1, HIDDEN], F32, tag="srow_part")
            nc.scalar.copy(srow_part[:], wsps[:])

            # ---------------- fp8 sign AllGather ----------------
            # Per-rank payload [129, 4096] u8: rows 0..127 = this core's fp8
            # signs laid out (p, c*1024+h); row 128 = the partial |W| column
            # sums as raw f32 bytes.  One collective shares everything.
            ag_in = dram_pool.tile([129, 4096], U8, name="ag_in")
            ag_out = dram_pool.tile([N_CORES, 129, 4096], U8,
                                    addr_space="Shared", name="ag_out")
            nc.sync.dma_start(
                ag_in[0:128, :].rearrange("p (c h) -> p c h", c=KTPC),
                wT8s[:].bitcast(U8))
            nc.sync.dma_start(ag_in[128:129, :].bitcast(F32), srow_part[:])
            nc.gpsimd.collective_compute(
                "AllGather", mybir.AluOpType.bypass,
                replica_groups=RG, ins=[ag_in.opt()], outs=[ag_out.opt()])

            # ------------- DMA dispatch, phase 2 (gpsimd SWDGE ring) -------------
            # Readback of the gathered signs, in the pair order DoubleRow
            # block b consumes them (ranks (b//4, b//4+4)); then the
            # remaining x tiles.  All of it sits behind the collective
            # trigger in the ring, which is exactly the priority we want.
            wT8 = wt_pool.tile([128, K_TILES, HIDDEN], FP8, tag="wT8",
                               name="wT8")
            for pair in range(KTPC):
                for r in (pair, pair + KTPC):
                    nc.gpsimd.dma_start(
                        wT8[:, r * KTPC:(r + 1) * KTPC, :].bitcast(U8),
                        ag_out[r, 0:128, :].rearrange(
                            "p (c h) -> p c h", c=KTPC))
            for m in range(2, M_TILES):
                dispatch_x(m)

            # res tiles ride the HWDGE (sync) queue: small, and off the
            # ring's critical path.
            for m in range(M_TILES):
                inp = res_pool.tile([128, HIDDEN], F32, tag="inp")
                nc.sync.dma_start(inp[:], res_ap[m * 128:(m + 1) * 128, :])
                inps[m] = inp

            # ---------------- x front / matmul / epilogue emitters ----------------
            def emit_x_front(m, x_fronts):
                xin = xins[m]
                xpackU = xpk_pool.tile([128, HALF], U16, tag="xpackU")
                _emit_pack(nc, xpk_pool, xin, xpackU, "xts")
                # transpose the 16 packed blocks on the PE (bit-exact for
                # the 4 sign-pair bf16 normals), staging through PSUM
                xTp = xt_pool.tile([128, NBLK, 128], U16, tag="xTp")
                for grp in range(2):
                    pst = pst_pool.tile([128, 8, 128], BF16, tag="pst")
                    for j in range(8):
                        blk = grp * 8 + j
                        nc.tensor.transpose(
                            pst[:, j, :],
                            xpackU[:, blk * 128:(blk + 1) * 128].bitcast(BF16),
                            ident[:])
                    nc.scalar.copy(
                        xTp[:, grp * 8:(grp + 1) * 8, :].bitcast(BF16),
                        pst[:])
                x_fronts[m] = xTp

            def emit_x_mms(m, x_fronts):
                xTp = x_fronts.pop(m)
                psum = psum_pool.tile([128, HIDDEN], F32, tag="psum",
                                      name="ps")
                for b in range(NBLK):
                    # forward interleaved byte-pairs; SwInterleave's column
                    # reversal is cancelled by the host-side row reversal
                    lhsT = xTp[:, b, :].bitcast(FP8)
                    for n in range(2):
                        nc.tensor.matmul(
                            psum[:, n * 512:(n + 1) * 512],
                            lhsT=lhsT,
                            rhs=wT8[:, b::NBLK, n * 512:(n + 1) * 512],
                            start=(b == 0), stop=(b == NBLK - 1),
                            perf_mode=DRSI)
                return psum

            def emit_epilogue(m, psum):
                # epilogue: r = psum * scaleF + inp (+ bB), then LayerNorm
                inp = inps[m]
                t = epi_pool.tile([128, HIDDEN], F32, tag="t")
                nc.vector.tensor_mul(t[:], psum[:], scaleF[:])
                r = epi_pool.tile([128, HIDDEN], F32, tag="r")
                nc.vector.tensor_add(r[:], t[:], inp[:])
                if use_b:
                    r2 = epi_pool.tile([128, HIDDEN], F32, tag="r2")
                    nc.vector.tensor_add(r2[:], r[:], bB[:])
                    r = r2

                bn6 = stats_pool.tile([128, 2, 6], F32, tag="bn6")
                nc.vector.bn_stats(bn6[:, 0, :], r[:, 0:512])
                nc.vector.bn_stats(bn6[:, 1, :], r[:, 512:1024])
                mv = stats_pool.tile([128, 2], F32, tag="mv")
                nc.vector.bn_aggr(mv[:], bn6[:])
                sd = stats_pool.tile([128, 1], F32, tag="sd")
                nc.scalar.activation(sd[:], mv[:, 1:2], AF.Sqrt,
                                     bias=epsT[:, 0:1])
                rstd = stats_pool.tile([128, 1], F32, tag="rstd")
                nc.vector.reciprocal(rstd[:], sd[:])
                nm = stats_pool.tile([128, 1], F32, tag="nm")
                nc.vector.tensor_scalar(out=nm[:], in0=mv[:, 0:1],
                                        scalar1=rstd[:, 0:1], scalar2=-1.0,
                                        op0=AT.mult, op1=AT.mult)
                y = epi_pool.tile([128, HIDDEN], F32, tag="y")
                nc.scalar.activation(y[:], r[:], AF.Identity,
                                     bias=nm[:, 0:1], scale=rstd[:, 0:1])
                if use_gamma:
                    y2 = epi_pool.tile([128, HIDDEN], F32, tag="y2")
                    nc.vector.tensor_mul(y2[:], y[:], gB[:])
                    y = y2
                if use_beta:
                    y3 = epi_pool.tile([128, HIDDEN], F32, tag="y3")
                    nc.vector.tensor_add(y3[:], y[:], beB[:])
                    y = y3

                nc.sync.dma_start(y_ap[m * 128:(m + 1) * 128, :], y[:])

            # Front the first two tiles so the PE has transpose work while
            # the collective is in flight.
            x_fronts = {}
            emit_x_front(0, x_fronts)
            emit_x_front(1, x_fronts)

            # ---------------- global scale row ----------------
            # The 8 partial rows rode the AllGather as raw f32 bytes; sum
            # them with one tiny fp32 matmul and broadcast via DRAM.
            spart = const_pool.tile([8, HIDDEN], F32, tag="spart")
            nc.sync.dma_start(
                spart[:],
                ag_out[:, 128, :].bitcast(F32))
            # same tag as wsps: reuses its PSUM banks (wsps is dead once
            # srow_part is copied out), keeping total PSUM within 8 banks
            psum_s = wsps_pool.tile([1, HIDDEN], F32, tag="wsps",
                                    name="psum_s")
            for n in range(2):
                nc.tensor.matmul(psum_s[:, n * 512:(n + 1) * 512],
                                 lhsT=ones8[:],
                                 rhs=spart[:, n * 512:(n + 1) * 512],
                                 start=True, stop=True)
            srow = const_pool.tile([1, HIDDEN], F32, tag="srow")
            nc.scalar.activation(srow[:], psum_s[:], AF.Copy,
                                 scale=float(scale_mul))
            scratch = dram_pool.tile([HIDDEN], F32)
            nc.sync.dma_start(
                out=scratch[:].rearrange("(a n) -> a n", a=1), in_=srow[:])
            scaleF = const_pool.tile([128, HIDDEN], F32, tag="scaleF")
            nc.sync.dma_start(
                scaleF[:],
                scratch[:].rearrange("(a n) -> a n", a=1).broadcast_to([128, HIDDEN]))

            bB = gB = beB = None
            if use_b:
                bB = const_pool.tile([128, HIDDEN], F32, tag="bB")
                nc.sync.dma_start(
                    bB[:],
                    b_ap.rearrange("(a n) -> a n", a=1).broadcast_to([128, HIDDEN]))
            if use_gamma:
                gB = const_pool.tile([128, HIDDEN], F32, tag="gB")
                nc.sync.dma_start(
                    gB[:],
                    g_ap.rearrange("(a n) -> a n", a=1).broadcast_to([128, HIDDEN]))
            if use_beta:
                beB = const_pool.tile([128, HIDDEN], F32, tag="beB")
                nc.sync.dma_start(
                    beB[:],
                    be_ap.rearrange("(a n) -> a n", a=1).broadcast_to([128, HIDDEN]))

            # ---------------- main loop over token tiles ----------------
            # Software-pipelined: tile m+2's pack/transpose (front) and tile
            # m-1's epilogue are emitted around tile m's matmuls, so the
            # in-order DVE/ACT/PE queues never stall the next tile's prep on
            # the previous tile's tail work.
            prev = None
            for m in range(M_TILES):
                psum = emit_x_mms(m, x_fronts)
                if m + 2 < M_TILES:
                    emit_x_front(m + 2, x_fronts)
                if prev is not None:
                    emit_epilogue(m - 1, prev)
                prev = psum
            emit_epilogue(M_TILES - 1, prev)


def _build(scale_mul: float, use_b: bool, use_gamma: bool, use_beta: bool):
    """Build the SPMD program (identical on all 8 cores).

    scale_mul = |clip_val| / INTER, folded into the per-channel scale.
    """
    nc = bacc.Bacc("TRN2", target_bir_lowering=False, debug=False,
                   num_devices=N_CORES)

    x_ap = nc.dram_tensor("x", [TPC, INTER], F32, kind="ExternalInput").ap()
    res_ap = nc.dram_tensor("res", [TPC, HIDDEN], F32, kind="ExternalInput").ap()
    wt_ap = nc.dram_tensor("wt", [KSLICE, HIDDEN], F32, kind="ExternalInput").ap()
    b_ap = g_ap = be_ap = None
    if use_b:
        b_ap = nc.dram_tensor("bvec", [HIDDEN], F32, kind="ExternalInput").ap()
    if use_gamma:
        g_ap = nc.dram_tensor("gvec", [HIDDEN], F32, kind="ExternalInput").ap()
    if use_beta:
        be_ap = nc.dram_tensor("bevec", [HIDDEN], F32, kind="ExternalInput").ap()
    y_ap = nc.dram_tensor("y", [TPC, HIDDEN], F32, kind="ExternalOutput").ap()

    _emit_program(nc, x_ap, res_ap, wt_ap, y_ap, b_ap, g_ap, be_ap,
                  scale_mul, use_b, use_gamma, use_beta)
    nc.compile()
    return nc


_last_results = None


def kernel(hidden_states, input_tensor, W, b, clip_val, gamma, beta):
    global _last_results
    hidden_states = np.asarray(hidden_states)
    input_tensor = np.asarray(input_tensor)
    W = np.asarray(W, dtype=np.float32)
    b = np.asarray(b, dtype=np.float32)
    gamma = np.asarray(gamma, dtype=np.float32)
    beta = np.asarray(beta, dtype=np.float32)
    clip = float(np.asarray(clip_val))

    use_b = bool(np.any(b != 0.0))
    use_gamma = bool(np.any(gamma != 1.0))
    use_beta = bool(np.any(beta != 0.0))
    scale_mul = abs(clip) / INTER

    key = (scale_mul, use_b, use_gamma, use_beta)
    if key not in _cache:
        _cache[key] = _build(scale_mul, use_b, use_gamma, use_beta)
    nc = _cache[key]

    hs = np.ascontiguousarray(
        hidden_states.reshape(TOKENS, INTER).astype(np.float32, copy=False))
    rs = np.ascontiguousarray(
        input_tensor.reshape(TOKENS, HIDDEN).astype(np.float32, copy=False))
    Wc = np.ascontiguousarray(W.T)   # layout choice: weight fed k-major

    in_maps = []
    for c in range(N_CORES):
        m = {
            "x": _prepare_x(hs[c * TPC:(c + 1) * TPC]),
            "res": np.ascontiguousarray(rs[c * TPC:(c + 1) * TPC]),
            "wt": np.ascontiguousarray(Wc[c * KSLICE:(c + 1) * KSLICE]),
        }
        if use_b:
            m["bvec"] = b
        if use_gamma:
            m["gvec"] = gamma
        if use_beta:
            m["bevec"] = beta
        in_maps.append(m)

    kwargs = {}
    if TRACE:
        _install_ntff_hook()
        kwargs["trace"] = True
        if TRACE_ALL_CORES:
            kwargs["trace_cores"] = list(range(N_CORES))
    res = bass_utils.run_bass_kernel_spmd(
        nc, in_maps, core_ids=list(range(N_CORES)), **kwargs)
    _last_results = res

    y = np.concatenate([res.results[c]["y"] for c in range(N_CORES)], axis=0)
    return y.reshape(hidden_states.shape[:-1] + (HIDDEN,)).astype(np.float32)


# revision 10
# speedup vs baseline: 1.0614x; 1.0614x over previous
"""Trainium2 Bass kernel for nn_BertOutput (binary-quantized BERT output layer).

Computation (see reference):
    w_scale = mean(|W|, axis=1)                  # [H, 1]
    W_q     = w_scale * sign(W)                  # [H, I]
    x_q     = clip * sign(x / clip)              # [B, S, I]
    h       = x_q @ W_q.T + b                    # [B, S, H]
    out     = LayerNorm(h + input_tensor) * gamma + beta

Structural facts exploited:
  * The matmul operands are exactly +-1: representable exactly in fp8e4m3,
    and the K=4096 accumulation of +-1 terms is exact in fp32 PSUM.  The
    per-output-channel scale (|clip| * mean|W|) is applied after the matmul.
  * fp8 enables MatmulPerfMode.DoubleRow: one instruction contracts TWO
    128-deep k-subtiles (157 TF/s peak), halving tensor-engine time vs bf16.
  * Sign bits survive the fp32->bf16 cast done during the DMA load.  x signs
    are packed PAIRWISE into u16 words -- fp8 sign of x[t, c] in the low
    byte, fp8 sign of x[t, 2048 + c] in the high byte -- with 3 contiguous
    DVE bitwise ops.  One 2-byte transpose then moves BOTH fp8 k-planes at
    once, and the byte-interleaved result is exactly what LDWEIGHTS perf
    mode DoubleRowSwInterleave consumes natively.  SwInterleave loads the
    first element to the largest PE column (reversing token order), which is
    cancelled by assigning tokens to SBUF partitions in reverse order when
    the shard is prepared on the host (a pure row permutation).
  * W is fed TRANSPOSED from the host (a pure layout/sharding choice), so
    it lands k-major and needs no on-device transpose.  It streams on the
    SWDGE ring in PAIR-GROUP order -- k-tile group g together with group
    g+4 -- because DoubleRow block b consumes k-tiles {b, b+16}: blocks
    4g..4g+3 become computable as soon as groups (g, g+4) are signed, while
    the rest of W is still in flight.
  * The matmul work is split: a PSUM-resident chunk (m-tiles 0-1) consumes
    the W pair-groups incrementally during the W stream (the accumulation
    order over k is free), and the remaining m-tiles run back-to-back once
    W is resident.  This removes the serial W-prep phase that previously
    idled the PE for the whole first half of the kernel.
  * The per-channel scale numerator sum_k |W[h,k]| is a cross-partition
    reduction in the W^T layout, computed as ones.T @ |W^T| on the PE;
    |w| tiles are pre-summed in pairs on the DVE to halve the PE matmuls.
  * DMA-xbar transposes lock ALL DMA engines for their whole duration (they
    cannot overlap the HBM loads), so the x tile transposes run on the PE
    array instead (is_transpose matmul; the packed u16 words are bf16
    normals, so a bf16 PE transpose is bit-exact, HW-verified).
  * Only the gpsimd ring can cast f32->bf16 in flight, and concurrent bulk
    on ring+sync queues CONTENDS (~339 GB/s aggregate vs ~390 single), so
    all bulk loads ride the ring in priority order and only the small res /
    output / broadcast traffic uses the sync queue.

Sharding: plain data-parallel over tokens -- 8192 tokens -> 1024 per core.
Each core computes a full LayerNorm over hidden=1024, so no collectives
(measured: the emulated 8-core AllGather costs ~50-60 us end-to-end due to
rank skew + mesh handshakes, which puts it on the critical path; sharing W
through it is a net loss).
"""

import sys

sys.path.insert(0, "/opt/trn_rl_repo")

import numpy as np

import concourse.bass as bass  # noqa: F401  (import side effects / registry)
import concourse.tile as tile
from concourse import bacc, bass_utils, mybir

F32 = mybir.dt.float32
BF16 = mybir.dt.bfloat16
FP8 = mybir.dt.float8e4
U16 = mybir.dt.uint16

HIDDEN = 1024
INTER = 4096
TOKENS = 8192
N_CORES = 8
TPC = TOKENS // N_CORES          # tokens per core = 1024
M_TILES = TPC // 128             # 8 token tiles per core
K_TILES = INTER // 128           # 32 k-tiles of W^T
W_GROUPS = 8                     # W streams as 8 groups of 4 k-tiles (2MB)
NBLK = INTER // 256              # 16 double-k-blocks (DoubleRow: 256 k each)
HALF = INTER // 2                # 2048: pack pairs (k, k + HALF)
A_TILES = 2                      # m-tiles accumulated during the W stream
EPS = 1e-12

TRACE = False                    # set True from test harness to profile
TRACE_ALL_CORES = False

_cache: dict = {}


def _install_ntff_hook():
    """The agent image's antenv package lacks axon_hooks, which silently
    disables NTFF profiling under axon.  Recreate it and wire the ctypes
    hook from trn_agent_boot (profiling/tooling only; the compute path
    does not depend on this)."""
    import types

    import antenv
    if getattr(antenv, "axon_hooks", None) is not None:
        return
    from trn_agent_boot.trn_boot import _ntff_profile_via_ctypes
    mod = types.ModuleType("antenv.axon_hooks")
    mod._hook = _ntff_profile_via_ctypes("/opt/axon/libaxon_pjrt.so")
    mod.get_axon_ntff_profile_hook = lambda: mod._hook

    def _set(h):
        mod._hook = h
    mod.set_axon_ntff_profile_hook = _set
    sys.modules["antenv.axon_hooks"] = mod
    antenv.axon_hooks = mod


def _prepare_x(x_shard: np.ndarray) -> np.ndarray:
    """Sharding-time row permutation: within each 128-token tile, tokens are
    assigned to SBUF partitions in REVERSE order, cancelling SwInterleave's
    first-element-to-largest-column reversal so psum rows come out natural."""
    t = x_shard.reshape(M_TILES, 128, INTER)
    return np.ascontiguousarray(t[:, ::-1, :]).reshape(TPC, INTER)


def _emit_pack(nc, pool, src, dst, tag):
    """Pack sign bits of a bf16 [128, 4096] tile into u16 fp8-sign pairs.

    dst u16 [128, 2048]: word c = lo byte fp8sign(src[:, c]),
                                  hi byte fp8sign(src[:, HALF + c]).
    fp8e4m3 +-1.0 is 0x38 / 0xB8, so:
        lo = (bf16_bits >> 8) & 0x0080  OR'd with  0x0038-from-tsB's 0x3838
        hi = (bf16_bits & 0x8000) | 0x3800
    """
    AT = mybir.AluOpType
    a = src[:, 0:HALF].bitcast(U16)
    b = src[:, HALF:INTER].bitcast(U16)
    tsA = pool.tile([128, HALF], U16, tag=f"{tag}A")
    nc.vector.tensor_scalar(out=tsA[:], in0=a, scalar1=8, scalar2=0x0080,
                            op0=AT.logical_shift_right, op1=AT.bitwise_and)
    tsB = pool.tile([128, HALF], U16, tag=f"{tag}B")
    nc.vector.tensor_scalar(out=tsB[:], in0=b, scalar1=0x8000, scalar2=0x3838,
                            op0=AT.bitwise_and, op1=AT.bitwise_or)
    nc.vector.tensor_tensor(out=dst[:], in0=tsA[:], in1=tsB[:],
                            op=AT.bitwise_or)


def _emit_program(nc, x_ap, res_ap, wt_ap, y_ap, b_ap, g_ap, be_ap,
                  scale_mul: float, use_b: bool, use_gamma: bool,
                  use_beta: bool):
    """Emit the per-core Tile program given DRAM APs.

    wt_ap is W TRANSPOSED ([INTER, HIDDEN]) -- a host-side layout choice so
    the weight lands k-major and needs no on-device transpose.
    """
    AT = mybir.AluOpType
    AF = mybir.ActivationFunctionType
    DRSI = mybir.MatmulPerfMode.DoubleRowSwInterleave
    from concourse.masks import make_identity

    with tile.TileContext(nc) as tc:
        with (
            tc.tile_pool(name="wt", bufs=1) as wt_pool,
            tc.tile_pool(name="wstage", bufs=3) as wstage_pool,
            tc.tile_pool(name="wpk", bufs=2) as wpk_pool,
            tc.tile_pool(name="const", bufs=1) as const_pool,
            tc.tile_pool(name="xio", bufs=3) as xio_pool,
            tc.tile_pool(name="xpk", bufs=2) as xpk_pool,
            tc.tile_pool(name="xt", bufs=3) as xt_pool,
            tc.tile_pool(name="res", bufs=8) as res_pool,
            tc.tile_pool(name="epi", bufs=2) as epi_pool,
            tc.tile_pool(name="stats", bufs=2) as stats_pool,
            tc.tile_pool(name="psum", bufs=2, space="PSUM") as psum_pool,
            tc.tile_pool(name="pst", bufs=2, space="PSUM") as pst_pool,
            tc.tile_pool(name="wsps", bufs=1, space="PSUM") as wsps_pool,
            tc.tile_pool(name="dram", bufs=1, space="DRAM") as dram_pool,
        ):
            epsT = const_pool.tile([128, 1], F32, tag="epsT")
            nc.vector.memset(epsT[:], float(EPS))
            ones1 = const_pool.tile([128, 1], BF16, tag="ones1")
            nc.vector.memset(ones1[:], 1.0)
            ident = const_pool.tile([128, 128], BF16, tag="ident")
            make_identity(nc, ident[:])

            # ---------------- DMA dispatch (gpsimd SWDGE ring) ----------------
            # The ring drains strictly in dispatch order, so it doubles as a
            # priority list: x0 first (feeds the transpose front), then the W
            # pair-groups (g, g+4) with x1/x2 interleaved, then the rest of x.
            xins, inps, wgs = {}, {}, {}

            def dispatch_x(m):
                xin = xio_pool.tile([128, INTER], BF16, tag="xin")
                nc.gpsimd.dma_start(xin[:], x_ap[m * 128:(m + 1) * 128, :])
                xins[m] = xin

            def dispatch_w(g):
                wg = wstage_pool.tile([128, 4, HIDDEN], BF16, tag="wld")
                nc.gpsimd.dma_start(
                    wg[:],
                    wt_ap[g * 512:(g + 1) * 512, :].rearrange(
                        "(c p) h -> p c h", p=128))
                wgs[g] = wg

            dispatch_x(0)
            dispatch_w(0)
            dispatch_w(4)
            dispatch_x(1)
            dispatch_w(1)
            dispatch_w(5)
            dispatch_x(2)
            for g in (2, 6, 3, 7):
                dispatch_w(g)
            for m in range(3, M_TILES):
                dispatch_x(m)

            # res tiles ride the HWDGE (sync) queue: small, and off the
            # ring's critical path.  All 8 are live (bufs=8), so none of
            # these DMAs ever waits on an epilogue.
            for m in range(M_TILES):
                inp = res_pool.tile([128, HIDDEN], F32, tag="inp")
                nc.sync.dma_start(inp[:], res_ap[m * 128:(m + 1) * 128, :])
                inps[m] = inp

            # ---------------- x front / matmul / epilogue emitters ----------------
            x_fronts = {}

            def emit_x_front(m):
                xin = xins[m]
                xpackU = xpk_pool.tile([128, HALF], U16, tag="xpackU")
                _emit_pack(nc, xpk_pool, xin, xpackU, "xts")
                # transpose the 16 packed blocks on the PE (bit-exact for
                # the 4 sign-pair bf16 normals), staging through PSUM
                xTp = xt_pool.tile([128, NBLK, 128], U16, tag="xTp")
                for grp in range(2):
                    pst = pst_pool.tile([128, 8, 128], BF16, tag="pst")
                    for j in range(8):
                        blk = grp * 8 + j
                        nc.tensor.transpose(
                            pst[:, j, :],
                            xpackU[:, blk * 128:(blk + 1) * 128].bitcast(BF16),
                            ident[:])
                    nc.scalar.copy(
                        xTp[:, grp * 8:(grp + 1) * 8, :].bitcast(BF16),
                        pst[:])
                x_fronts[m] = xTp

            def emit_block_mms(psum, xTp, b, start, stop):
                # forward interleaved byte-pairs; SwInterleave's column
                # reversal is cancelled by the host-side row reversal
                lhsT = xTp[:, b, :].bitcast(FP8)
                for n in range(2):
                    nc.tensor.matmul(
                        psum[:, n * 512:(n + 1) * 512],
                        lhsT=lhsT,
                        rhs=wT8[:, b::NBLK, n * 512:(n + 1) * 512],
                        start=start, stop=stop,
                        perf_mode=DRSI)

            def emit_x_mms(m):
                xTp = x_fronts.pop(m)
                psum = psum_pool.tile([128, HIDDEN], F32, tag="psum",
                                      name="ps")
                for b in range(NBLK):
                    emit_block_mms(psum, xTp, b, b == 0, b == NBLK - 1)
                return psum

            def emit_epilogue(m, src):
                # epilogue: r = src * scaleF + inp (+ bB), then LayerNorm.
                # src is either a PSUM tile or its SBUF copy (chunk A).
                inp = inps[m]
                t = epi_pool.tile([128, HIDDEN], F32, tag="t")
                nc.vector.tensor_mul(t[:], src[:], scaleF[:])
                r = epi_pool.tile([128, HIDDEN], F32, tag="r")
                nc.vector.tensor_add(r[:], t[:], inp[:])
                if use_b:
                    r2 = epi_pool.tile([128, HIDDEN], F32, tag="r2")
                    nc.vector.tensor_add(r2[:], r[:], bB[:])
                    r = r2

                bn6 = stats_pool.tile([128, 2, 6], F32, tag="bn6")
                nc.vector.bn_stats(bn6[:, 0, :], r[:, 0:512])
                nc.vector.bn_stats(bn6[:, 1, :], r[:, 512:1024])
                mv = stats_pool.tile([128, 2], F32, tag="mv")
                nc.vector.bn_aggr(mv[:], bn6[:])
                sd = stats_pool.tile([128, 1], F32, tag="sd")
                nc.scalar.activation(sd[:], mv[:, 1:2], AF.Sqrt,
                                     bias=epsT[:, 0:1])
                rstd = stats_pool.tile([128, 1], F32, tag="rstd")
                nc.vector.reciprocal(rstd[:], sd[:])
                nm = stats_pool.tile([128, 1], F32, tag="nm")
                nc.vector.tensor_scalar(out=nm[:], in0=mv[:, 0:1],
                                        scalar1=rstd[:, 0:1], scalar2=-1.0,
                                        op0=AT.mult, op1=AT.mult)
                y = epi_pool.tile([128, HIDDEN], F32, tag="y")
                nc.scalar.activation(y[:], r[:], AF.Identity,
                                     bias=nm[:, 0:1], scale=rstd[:, 0:1])
                if use_gamma:
                    y2 = epi_pool.tile([128, HIDDEN], F32, tag="y2")
                    nc.vector.tensor_mul(y2[:], y[:], gB[:])
                    y = y2
                if use_beta:
                    y3 = epi_pool.tile([128, HIDDEN], F32, tag="y3")
                    nc.vector.tensor_add(y3[:], y[:], beB[:])
                    y = y3

                nc.sync.dma_start(y_ap[m * 128:(m + 1) * 128, :], y[:])

            # ---------------- W prep + chunk A (during the W stream) -------
            # wT8 fp8 [128, 32, 1024]: (p, kt, h) = fp8 sign W[h, kt*128+p].
            # DoubleRow rhs block b, half n = [:, b::16, n*512:(n+1)*512]
            # (k-pair (b, b+16) matches the x pack pairing (c, 2048+c)).
            wT8 = wt_pool.tile([128, K_TILES, HIDDEN], FP8, tag="wT8",
                               name="wT8")
            wsps = wsps_pool.tile([1, HIDDEN], F32, tag="wsps", name="wsps")

            emit_x_front(0)
            psumA = []
            for m in range(A_TILES):
                psumA.append(psum_pool.tile([128, HIDDEN], F32, tag="psum",
                                            name="psA"))

            for gp in range(4):
                for c in range(4):
                    # sign both halves of the k-pair; alternate engines so
                    # the scalar and vector queues split the work
                    for i, g in enumerate((gp, gp + 4)):
                        kt = g * 4 + c
                        wld = wgs[g]
                        if (c + i) % 2 == 1:
                            wsg = wpk_pool.tile([128, HIDDEN], BF16,
                                                tag="wsg")
                            nc.vector.tensor_scalar(
                                out=wsg[:].bitcast(U16),
                                in0=wld[:, c, :].bitcast(U16),
                                scalar1=0x8000, scalar2=0x3F80,
                                op0=AT.bitwise_and, op1=AT.bitwise_or)
                            nc.vector.tensor_copy(wT8[:, kt, :], wsg[:])
                        else:
                            nc.scalar.sign(wT8[:, kt, :], wld[:, c, :])
                    # |w| of both halves via sign-bit mask, pair-sum on the
                    # DVE, then ones.T @ (|w_lo|+|w_hi|) accumulates the
                    # per-channel scale numerator on the PE
                    wabs = []
                    for g in (gp, gp + 4):
                        wa = wpk_pool.tile([128, HIDDEN], BF16, tag="wabs")
                        nc.vector.tensor_scalar(
                            out=wa[:].bitcast(U16),
                            in0=wgs[g][:, c, :].bitcast(U16),
                            scalar1=0x7FFF, scalar2=None,
                            op0=AT.bitwise_and)
                        wabs.append(wa)
                    wps = wpk_pool.tile([128, HIDDEN], BF16, tag="wps")
                    nc.vector.tensor_add(wps[:], wabs[0][:], wabs[1][:])
                    for n in range(2):
                        nc.tensor.matmul(wsps[:, n * 512:(n + 1) * 512],
                                         lhsT=ones1[:],
                                         rhs=wps[:, n * 512:(n + 1) * 512],
                                         start=(gp == 0 and c == 0),
                                         stop=(gp == 3 and c == 3))
                # chunk A: m0 consumes this pair-group's blocks immediately;
                # m1 trails one group behind (its front is emitted during
                # group 0, so it has no transposed x yet at gp == 0) and
                # catches up after the loop.
                for b in range(gp * 4, gp * 4 + 4):
                    emit_block_mms(psumA[0], x_fronts[0], b,
                                   b == 0, b == NBLK - 1)
                if gp >= 1:
                    for b in range((gp - 1) * 4, gp * 4):
                        emit_block_mms(psumA[1], x_fronts[1], b,
                                       b == 0, False)
                if gp == 0:
                    emit_x_front(1)
                if gp == 1:
                    emit_x_front(2)
            for b in range(12, NBLK):
                emit_block_mms(psumA[1], x_fronts[1], b, False, b == NBLK - 1)

            # ---------------- per-channel scale + broadcasts ----------------
            srow = const_pool.tile([1, HIDDEN], F32, tag="srow")
            nc.scalar.activation(srow[:], wsps[:], AF.Copy,
                                 scale=float(scale_mul))
            scratch = dram_pool.tile([HIDDEN], F32)
            nc.sync.dma_start(
                out=scratch[:].rearrange("(a n) -> a n", a=1), in_=srow[:])
            scaleF = const_pool.tile([128, HIDDEN], F32, tag="scaleF")
            nc.sync.dma_start(
                scaleF[:],
                scratch[:].rearrange("(a n) -> a n", a=1).broadcast_to([128, HIDDEN]))

            bB = gB = beB = None
            if use_b:
                bB = const_pool.tile([128, HIDDEN], F32, tag="bB")
                nc.sync.dma_start(
                    bB[:],
                    b_ap.rearrange("(a n) -> a n", a=1).broadcast_to([128, HIDDEN]))
            if use_gamma:
                gB = const_pool.tile([128, HIDDEN], F32, tag="gB")
                nc.sync.dma_start(
                    gB[:],
                    g_ap.rearrange("(a n) -> a n", a=1).broadcast_to([128, HIDDEN]))
            if use_beta:
                beB = const_pool.tile([128, HIDDEN], F32, tag="beB")
                nc.sync.dma_start(
                    beB[:],
                    be_ap.rearrange("(a n) -> a n", a=1).broadcast_to([128, HIDDEN]))

            # Copy chunk A psums to SBUF immediately: frees their PSUM banks
            # for the tail loop, and breaks the scaleF <-> psum-slot cycle
            # (the epilogue can then wait for scaleF without holding PSUM).
            psA_sb = []
            for m in range(A_TILES):
                sb = epi_pool.tile([128, HIDDEN], F32, tag="psb")
                nc.vector.tensor_copy(sb[:], psumA[m][:])
                psA_sb.append(sb)
            x_fronts.pop(0)
            x_fronts.pop(1)

            emit_x_front(3)
            for m in range(A_TILES):
                emit_epilogue(m, psA_sb[m])

            # ---------------- tail loop over remaining m-tiles ----------------
            # Software-pipelined exactly like the baseline: tile m+2's
            # pack/transpose and tile m-1's epilogue are emitted around tile
            # m's matmuls so the in-order engine queues never stall.
            prev = None
            prev_m = None
            for m in range(A_TILES, M_TILES):
                psum = emit_x_mms(m)
                if m + 2 < M_TILES:
                    emit_x_front(m + 2)
                if prev is not None:
                    emit_epilogue(prev_m, prev)
                prev, prev_m = psum, m
            emit_epilogue(prev_m, prev)


def _build(scale_mul: float, use_b: bool, use_gamma: bool, use_beta: bool):
    """Build the SPMD program (identical on all 8 cores).

    scale_mul = |clip_val| / INTER, folded into the per-channel scale.
    """
    nc = bacc.Bacc("TRN2", target_bir_lowering=False, debug=False,
                   num_devices=N_CORES)

    x_ap = nc.dram_tensor("x", [TPC, INTER], F32, kind="ExternalInput").ap()
    res_ap = nc.dram_tensor("res", [TPC, HIDDEN], F32, kind="ExternalInput").ap()
    wt_ap = nc.dram_tensor("wt", [INTER, HIDDEN], F32, kind="ExternalInput").ap()
    b_ap = g_ap = be_ap = None
    if use_b:
        b_ap = nc.dram_tensor("bvec", [HIDDEN], F32, kind="ExternalInput").ap()
    if use_gamma:
        g_ap = nc.dram_tensor("gvec", [HIDDEN], F32, kind="ExternalInput").ap()
    if use_beta:
        be_ap = nc.dram_tensor("bevec", [HIDDEN], F32, kind="ExternalInput").ap()
    y_ap = nc.dram_tensor("y", [TPC, HIDDEN], F32, kind="ExternalOutput").ap()

    _emit_program(nc, x_ap, res_ap, wt_ap, y_ap, b_ap, g_ap, be_ap,
                  scale_mul, use_b, use_gamma, use_beta)
    nc.compile()
    return nc


_last_results = None


def kernel(hidden_states, input_tensor, W, b, clip_val, gamma, beta):
    global _last_results
    hidden_states = np.asarray(hidden_states)
    input_tensor = np.asarray(input_tensor)
    W = np.asarray(W, dtype=np.float32)
    b = np.asarray(b, dtype=np.float32)
    gamma = np.asarray(gamma, dtype=np.float32)
    beta = np.asarray(beta, dtype=np.float32)
    clip = float(np.asarray(clip_val))

    use_b = bool(np.any(b != 0.0))
    use_gamma = bool(np.any(gamma != 1.0))
    use_beta = bool(np.any(beta != 0.0))
    scale_mul = abs(clip) / INTER

    key = (scale_mul, use_b, use_gamma, use_beta)
    if key not in _cache:
        _cache[key] = _build(scale_mul, use_b, use_gamma, use_beta)
    nc = _cache[key]

    hs = np.ascontiguousarray(
        hidden_states.reshape(TOKENS, INTER).astype(np.float32, copy=False))
    rs = np.ascontiguousarray(
        input_tensor.reshape(TOKENS, HIDDEN).astype(np.float32, copy=False))
    Wc = np.ascontiguousarray(W.T)   # layout choice: weight fed k-major

    in_maps = []
    for c in range(N_CORES):
        m = {
            "x": _prepare_x(hs[c * TPC:(c + 1) * TPC]),
            "res": np.ascontiguousarray(rs[c * TPC:(c + 1) * TPC]),
            "wt": Wc,
        }
        if use_b:
            m["bvec"] = b
        if use_gamma:
            m["gvec"] = gamma
        if use_beta:
            m["bevec"] = beta
        in_maps.append(m)

    kwargs = {}
    if TRACE:
        _install_ntff_hook()
        kwargs["trace"] = True
        if TRACE_ALL_CORES:
            kwargs["trace_cores"] = list(range(N_CORES))
    res = bass_utils.run_bass_kernel_spmd(
        nc, in_maps, core_ids=list(range(N_CORES)), **kwargs)
    _last_results = res

    y = np.concatenate([res.results[c]["y"] for c in range(N_CORES)], axis=0)
    return y.reshape(hidden_states.shape[:-1] + (HIDDEN,)).astype(np.float32)


# revision 13
# speedup vs baseline: 1.1799x; 1.1116x over previous
"""Trainium2 Bass kernel for nn_BertOutput (binary-quantized BERT output layer).

Computation (see reference):
    w_scale = mean(|W|, axis=1)                  # [H, 1]
    W_q     = w_scale * sign(W)                  # [H, I]
    x_q     = clip * sign(x / clip)              # [B, S, I]
    h       = x_q @ W_q.T + b                    # [B, S, H]
    out     = LayerNorm(h + input_tensor) * gamma + beta

Structural facts exploited:
  * The matmul operands are exactly +-1: representable exactly in fp8e4m3,
    and the K=4096 accumulation of +-1 terms is exact in fp32 PSUM.  The
    per-output-channel scale (|clip| * mean|W|) is applied after the matmul.
  * fp8 enables MatmulPerfMode.DoubleRow: one instruction contracts TWO
    128-deep k-subtiles (157 TF/s peak), halving tensor-engine time vs bf16.
  * Sign bits survive the fp32->bf16 cast done during the DMA load.  x signs
    are packed PAIRWISE into u16 words -- fp8 sign of x[t, c] in the low
    byte, fp8 sign of x[t, 2048 + c] in the high byte -- with 3 contiguous
    DVE bitwise ops.  One 2-byte transpose then moves BOTH fp8 k-planes at
    once, and the byte-interleaved result is exactly what LDWEIGHTS perf
    mode DoubleRowSwInterleave consumes natively.  SwInterleave loads the
    first element to the largest PE column (reversing token order), which is
    cancelled by assigning tokens to SBUF partitions in reverse order when
    the shard is prepared on the host (a pure row permutation).
  * W is fed TRANSPOSED from the host (a pure layout/sharding choice), so
    it lands k-major and needs no on-device transpose.  It streams on the
    SWDGE ring in PAIR-GROUP order -- k-tile group g together with group
    g+4 -- because DoubleRow block b consumes k-tiles {b, b+16}: blocks
    4g..4g+3 become computable as soon as groups (g, g+4) are signed, while
    the rest of W is still in flight.
  * The matmul work is split: a PSUM-resident chunk (m-tiles 0-1) consumes
    the W pair-groups incrementally during the W stream (the accumulation
    order over k is free), and the remaining m-tiles run back-to-back once
    W is resident.  This removes the serial W-prep phase that previously
    idled the PE for the whole first half of the kernel.
  * The per-channel scale numerator sum_k |W[h,k]| is a cross-partition
    reduction in the W^T layout, computed as ones.T @ |W^T| on the PE;
    |w| tiles are pre-summed in pairs on the DVE to halve the PE matmuls.
  * DMA-xbar transposes lock ALL DMA engines for their whole duration (they
    cannot overlap the HBM loads), so the x tile transposes run on the PE
    array instead (is_transpose matmul; the packed u16 words are bf16
    normals, so a bf16 PE transpose is bit-exact, HW-verified).
  * Only the gpsimd ring can cast f32->bf16 in flight, and concurrent bulk
    on ring+sync queues CONTENDS (~339 GB/s aggregate vs ~390 single), so
    all bulk loads ride the ring in priority order and only the small res /
    output / broadcast traffic uses the sync queue.

Sharding: plain data-parallel over tokens -- 8192 tokens -> 1024 per core.
Each core computes a full LayerNorm over hidden=1024, so no collectives
(measured: the emulated 8-core AllGather costs ~50-60 us end-to-end due to
rank skew + mesh handshakes, which puts it on the critical path; sharing W
through it is a net loss).
"""

import sys

sys.path.insert(0, "/opt/trn_rl_repo")

import numpy as np

import concourse.bass as bass  # noqa: F401  (import side effects / registry)
import concourse.tile as tile
from concourse import bacc, bass_utils, mybir

F32 = mybir.dt.float32
BF16 = mybir.dt.bfloat16
FP8 = mybir.dt.float8e4
U16 = mybir.dt.uint16

HIDDEN = 1024
INTER = 4096
TOKENS = 8192
N_CORES = 8
TPC = TOKENS // N_CORES          # tokens per core = 1024
M_TILES = TPC // 128             # 8 token tiles per core
K_TILES = INTER // 128           # 32 k-tiles of W^T
W_GROUPS = 8                     # W streams as 8 groups of 4 k-tiles (2MB)
NBLK = INTER // 256              # 16 double-k-blocks (DoubleRow: 256 k each)
HALF = INTER // 2                # 2048: pack pairs (k, k + HALF)
A_TILES = 2                      # m-tiles accumulated during the W stream
EPS = 1e-12

TRACE = False                    # set True from test harness to profile
TRACE_ALL_CORES = False

_cache: dict = {}


def _install_ntff_hook():
    """The agent image's antenv package lacks axon_hooks, which silently
    disables NTFF profiling under axon.  Recreate it and wire the ctypes
    hook from trn_agent_boot (profiling/tooling only; the compute path
    does not depend on this)."""
    import types

    import antenv
    if getattr(antenv, "axon_hooks", None) is not None:
        return
    from trn_agent_boot.trn_boot import _ntff_profile_via_ctypes
    mod = types.ModuleType("antenv.axon_hooks")
    mod._hook = _ntff_profile_via_ctypes("/opt/axon/libaxon_pjrt.so")
    mod.get_axon_ntff_profile_hook = lambda: mod._hook

    def _set(h):
        mod._hook = h
    mod.set_axon_ntff_profile_hook = _set
    sys.modules["antenv.axon_hooks"] = mod
    antenv.axon_hooks = mod


def _prepare_x(x_shard: np.ndarray) -> np.ndarray:
    """Sharding-time row permutation: within each 128-token tile, tokens are
    assigned to SBUF partitions in REVERSE order, cancelling SwInterleave's
    first-element-to-largest-column reversal so psum rows come out natural."""
    t = x_shard.reshape(M_TILES, 128, INTER)
    return np.ascontiguousarray(t[:, ::-1, :]).reshape(TPC, INTER)


def _emit_pack(nc, pool, src, dst, tag):
    """Pack sign bits of a bf16 [128, 4096] tile into u16 fp8-sign pairs.

    dst u16 [128, 2048]: word c = lo byte fp8sign(src[:, c]),
                                  hi byte fp8sign(src[:, HALF + c]).
    fp8e4m3 +-1.0 is 0x38 / 0xB8, so:
        lo = (bf16_bits >> 8) & 0x0080  OR'd with  0x0038-from-tsB's 0x3838
        hi = (bf16_bits & 0x8000) | 0x3800
    """
    AT = mybir.AluOpType
    a = src[:, 0:HALF].bitcast(U16)
    b = src[:, HALF:INTER].bitcast(U16)
    tsA = pool.tile([128, HALF], U16, tag=f"{tag}A", bufs=1)
    nc.vector.tensor_scalar(out=tsA[:], in0=a, scalar1=8, scalar2=0x0080,
                            op0=AT.logical_shift_right, op1=AT.bitwise_and)
    tsB = pool.tile([128, HALF], U16, tag=f"{tag}B", bufs=1)
    nc.vector.tensor_scalar(out=tsB[:], in0=b, scalar1=0x8000, scalar2=0x3838,
                            op0=AT.bitwise_and, op1=AT.bitwise_or)
    nc.vector.tensor_tensor(out=dst[:], in0=tsA[:], in1=tsB[:],
                            op=AT.bitwise_or)


def _emit_program(nc, x_ap, res_ap, wt_ap, y_ap, b_ap, g_ap, be_ap,
                  scale_mul: float, use_b: bool, use_gamma: bool,
                  use_beta: bool):
    """Emit the per-core Tile program given DRAM APs.

    wt_ap is W TRANSPOSED ([INTER, HIDDEN]) -- a host-side layout choice so
    the weight lands k-major and needs no on-device transpose.
    """
    AT = mybir.AluOpType
    AF = mybir.ActivationFunctionType
    DRSI = mybir.MatmulPerfMode.DoubleRowSwInterleave
    from concourse.masks import make_identity

    with tile.TileContext(nc) as tc:
        with (
            tc.tile_pool(name="wt", bufs=1) as wt_pool,
            tc.tile_pool(name="wstage", bufs=4) as wstage_pool,
            tc.tile_pool(name="wpk", bufs=2) as wpk_pool,
            tc.tile_pool(name="const", bufs=1) as const_pool,
            tc.tile_pool(name="xio", bufs=3) as xio_pool,
            tc.tile_pool(name="xpk", bufs=2) as xpk_pool,
            tc.tile_pool(name="xt", bufs=3) as xt_pool,
            tc.tile_pool(name="res", bufs=8) as res_pool,
            tc.tile_pool(name="epi", bufs=2) as epi_pool,
            tc.tile_pool(name="stats", bufs=2) as stats_pool,
            tc.tile_pool(name="psum", bufs=2, space="PSUM") as psum_pool,
            tc.tile_pool(name="pst", bufs=2, space="PSUM") as pst_pool,
            tc.tile_pool(name="wsps", bufs=1, space="PSUM") as wsps_pool,
            tc.tile_pool(name="dram", bufs=1, space="DRAM") as dram_pool,
        ):
            epsT = const_pool.tile([128, 1], F32, tag="epsT")
            nc.vector.memset(epsT[:], float(EPS))
            ones1 = const_pool.tile([128, 1], BF16, tag="ones1")
            nc.vector.memset(ones1[:], 1.0)
            ident = const_pool.tile([128, 128], BF16, tag="ident")
            make_identity(nc, ident[:])

            # ---------------- DMA dispatch (gpsimd SWDGE ring) ----------------
            # The ring drains strictly in dispatch order, so it doubles as a
            # priority list: x0 first (feeds the transpose front), then the W
            # pair-groups (g, g+4) with x1/x2 interleaved, then the rest of x.
            xins, inps, wgs = {}, {}, {}

            def dispatch_x(m):
                xin = xio_pool.tile([128, INTER], BF16, tag="xin")
                nc.gpsimd.dma_start(xin[:], x_ap[m * 128:(m + 1) * 128, :])
                xins[m] = xin

            def dispatch_w(g):
                wg = wstage_pool.tile([128, 4, HIDDEN], BF16, tag="wld")
                nc.gpsimd.dma_start(
                    wg[:],
                    wt_ap[g * 512:(g + 1) * 512, :].rearrange(
                        "(c p) h -> p c h", p=128))
                wgs[g] = wg

            dispatch_x(0)
            dispatch_w(0)
            dispatch_w(4)
            dispatch_x(1)
            dispatch_w(1)
            dispatch_w(5)
            dispatch_x(2)
            for g in (2, 6, 3, 7):
                dispatch_w(g)
            for m in range(3, M_TILES):
                dispatch_x(m)

            # res tiles ride the HWDGE (sync) queue: small, and off the
            # ring's critical path.  All 8 are live (bufs=8), so none of
            # these DMAs ever waits on an epilogue.
            for m in range(M_TILES):
                inp = res_pool.tile([128, HIDDEN], F32, tag="inp")
                nc.sync.dma_start(inp[:], res_ap[m * 128:(m + 1) * 128, :])
                inps[m] = inp

            # ---------------- x front / matmul / epilogue emitters ----------------
            x_fronts = {}

            def emit_x_front(m):
                xin = xins[m]
                xpackU = xpk_pool.tile([128, HALF], U16, tag="xpackU")
                _emit_pack(nc, xpk_pool, xin, xpackU, "xts")
                # transpose the 16 packed blocks on the PE (bit-exact for
                # the 4 sign-pair bf16 normals), staging through PSUM
                xTp = xt_pool.tile([128, NBLK, 128], U16, tag="xTp")
                for grp in range(2):
                    pst = pst_pool.tile([128, 8, 128], BF16, tag="pst")
                    for j in range(8):
                        blk = grp * 8 + j
                        nc.tensor.transpose(
                            pst[:, j, :],
                            xpackU[:, blk * 128:(blk + 1) * 128].bitcast(BF16),
                            ident[:])
                    nc.scalar.copy(
                        xTp[:, grp * 8:(grp + 1) * 8, :].bitcast(BF16),
                        pst[:])
                x_fronts[m] = xTp

            def emit_block_mms(psum, xTp, b, start, stop):
                # forward interleaved byte-pairs; SwInterleave's column
                # reversal is cancelled by the host-side row reversal
                lhsT = xTp[:, b, :].bitcast(FP8)
                for n in range(2):
                    nc.tensor.matmul(
                        psum[:, n * 512:(n + 1) * 512],
                        lhsT=lhsT,
                        rhs=wT8[:, b::NBLK, n * 512:(n + 1) * 512],
                        start=start, stop=stop,
                        perf_mode=DRSI)

            def emit_x_mms(m):
                xTp = x_fronts.pop(m)
                psum = psum_pool.tile([128, HIDDEN], F32, tag="psum",
                                      name="ps")
                for b in range(NBLK):
                    emit_block_mms(psum, xTp, b, b == 0, b == NBLK - 1)
                return psum

            def emit_epilogue(m, src):
                # epilogue: r = src * scaleF + inp (+ bB), then LayerNorm.
                # src is either a PSUM tile or its SBUF copy (chunk A).
                inp = inps[m]
                t = epi_pool.tile([128, HIDDEN], F32, tag="t")
                nc.vector.tensor_mul(t[:], src[:], scaleF[:])
                # in-place accumulate of the residual (saves an 8KB epi tag)
                nc.vector.tensor_add(t[:], t[:], inp[:])
                r = t
                if use_b:
                    r2 = epi_pool.tile([128, HIDDEN], F32, tag="r2")
                    nc.vector.tensor_add(r2[:], r[:], bB[:])
                    r = r2

                bn6 = stats_pool.tile([128, 2, 6], F32, tag="bn6")
                nc.vector.bn_stats(bn6[:, 0, :], r[:, 0:512])
                nc.vector.bn_stats(bn6[:, 1, :], r[:, 512:1024])
                mv = stats_pool.tile([128, 2], F32, tag="mv")
                nc.vector.bn_aggr(mv[:], bn6[:])
                sd = stats_pool.tile([128, 1], F32, tag="sd")
                nc.scalar.activation(sd[:], mv[:, 1:2], AF.Sqrt,
                                     bias=epsT[:, 0:1])
                rstd = stats_pool.tile([128, 1], F32, tag="rstd")
                nc.vector.reciprocal(rstd[:], sd[:])
                nm = stats_pool.tile([128, 1], F32, tag="nm")
                nc.vector.tensor_scalar(out=nm[:], in0=mv[:, 0:1],
                                        scalar1=rstd[:, 0:1], scalar2=-1.0,
                                        op0=AT.mult, op1=AT.mult)
                y = epi_pool.tile([128, HIDDEN], F32, tag="y")
                nc.scalar.activation(y[:], r[:], AF.Identity,
                                     bias=nm[:, 0:1], scale=rstd[:, 0:1])
                if use_gamma:
                    y2 = epi_pool.tile([128, HIDDEN], F32, tag="y2")
                    nc.vector.tensor_mul(y2[:], y[:], gB[:])
                    y = y2
                if use_beta:
                    y3 = epi_pool.tile([128, HIDDEN], F32, tag="y3")
                    nc.vector.tensor_add(y3[:], y[:], beB[:])
                    y = y3

                nc.sync.dma_start(y_ap[m * 128:(m + 1) * 128, :], y[:])

            # ---------------- W prep + chunk A (during the W stream) -------
            # wT8 fp8 [128, 32, 1024]: (p, kt, h) = fp8 sign W[h, kt*128+p].
            # DoubleRow rhs block b, half n = [:, b::16, n*512:(n+1)*512]
            # (k-pair (b, b+16) matches the x pack pairing (c, 2048+c)).
            wT8 = wt_pool.tile([128, K_TILES, HIDDEN], FP8, tag="wT8",
                               name="wT8")
            wsps = wsps_pool.tile([1, HIDDEN], F32, tag="wsps", name="wsps")

            emit_x_front(0)
            psumA = []
            for m in range(A_TILES):
                psumA.append(psum_pool.tile([128, HIDDEN], F32, tag="psum",
                                            name="psA"))

            for gp in range(4):
                for c in range(4):
                    # sign both halves of the k-pair; alternate engines so
                    # the scalar and vector queues split the work
                    for i, g in enumerate((gp, gp + 4)):
                        kt = g * 4 + c
                        wld = wgs[g]
                        if (c + i) % 2 == 1:
                            wsg = wpk_pool.tile([128, HIDDEN], BF16,
                                                tag="wsg")
                            nc.vector.tensor_scalar(
                                out=wsg[:].bitcast(U16),
                                in0=wld[:, c, :].bitcast(U16),
                                scalar1=0x8000, scalar2=0x3F80,
                                op0=AT.bitwise_and, op1=AT.bitwise_or)
                            nc.vector.tensor_copy(wT8[:, kt, :], wsg[:])
                        else:
                            nc.scalar.sign(wT8[:, kt, :], wld[:, c, :])
                    # |w| of both halves via sign-bit mask, pair-sum on the
                    # DVE, then ones.T @ (|w_lo|+|w_hi|) accumulates the
                    # per-channel scale numerator on the PE
                    wabs = []
                    for g in (gp, gp + 4):
                        wa = wpk_pool.tile([128, HIDDEN], BF16, tag="wabs")
                        nc.vector.tensor_scalar(
                            out=wa[:].bitcast(U16),
                            in0=wgs[g][:, c, :].bitcast(U16),
                            scalar1=0x7FFF, scalar2=None,
                            op0=AT.bitwise_and)
                        wabs.append(wa)
                    wps = wpk_pool.tile([128, HIDDEN], BF16, tag="wps")
                    nc.vector.tensor_add(wps[:], wabs[0][:], wabs[1][:])
                    for n in range(2):
                        nc.tensor.matmul(wsps[:, n * 512:(n + 1) * 512],
                                         lhsT=ones1[:],
                                         rhs=wps[:, n * 512:(n + 1) * 512],
                                         start=(gp == 0 and c == 0),
                                         stop=(gp == 3 and c == 3))
                # chunk A: m0 consumes this pair-group's blocks immediately;
                # m1 trails one group behind (its front is emitted during
                # group 0, so it has no transposed x yet at gp == 0) and
                # catches up after the loop.
                for b in range(gp * 4, gp * 4 + 4):
                    emit_block_mms(psumA[0], x_fronts[0], b,
                                   b == 0, b == NBLK - 1)
                if gp >= 1:
                    for b in range((gp - 1) * 4, gp * 4):
                        emit_block_mms(psumA[1], x_fronts[1], b,
                                       b == 0, False)
                if gp == 0:
                    emit_x_front(1)
                if gp == 1:
                    emit_x_front(2)
            for b in range(12, NBLK):
                emit_block_mms(psumA[1], x_fronts[1], b, False, b == NBLK - 1)

            # ---------------- per-channel scale + broadcasts ----------------
            srow = const_pool.tile([1, HIDDEN], F32, tag="srow")
            nc.scalar.activation(srow[:], wsps[:], AF.Copy,
                                 scale=float(scale_mul))
            scratch = dram_pool.tile([HIDDEN], F32)
            nc.sync.dma_start(
                out=scratch[:].rearrange("(a n) -> a n", a=1), in_=srow[:])
            scaleF = const_pool.tile([128, HIDDEN], F32, tag="scaleF")
            nc.sync.dma_start(
                scaleF[:],
                scratch[:].rearrange("(a n) -> a n", a=1).broadcast_to([128, HIDDEN]))

            bB = gB = beB = None
            if use_b:
                bB = const_pool.tile([128, HIDDEN], F32, tag="bB")
                nc.sync.dma_start(
                    bB[:],
                    b_ap.rearrange("(a n) -> a n", a=1).broadcast_to([128, HIDDEN]))
            if use_gamma:
                gB = const_pool.tile([128, HIDDEN], F32, tag="gB")
                nc.sync.dma_start(
                    gB[:],
                    g_ap.rearrange("(a n) -> a n", a=1).broadcast_to([128, HIDDEN]))
            if use_beta:
                beB = const_pool.tile([128, HIDDEN], F32, tag="beB")
                nc.sync.dma_start(
                    beB[:],
                    be_ap.rearrange("(a n) -> a n", a=1).broadcast_to([128, HIDDEN]))

            # Copy chunk A psums to SBUF immediately: frees their PSUM banks
            # for the tail loop, and breaks the scaleF <-> psum-slot cycle
            # (the epilogue can then wait for scaleF without holding PSUM).
            psA_sb = []
            for m in range(A_TILES):
                sb = epi_pool.tile([128, HIDDEN], F32, tag="psb")
                nc.vector.tensor_copy(sb[:], psumA[m][:])
                psA_sb.append(sb)
            x_fronts.pop(0)
            x_fronts.pop(1)

            emit_x_front(3)
            for m in range(A_TILES):
                emit_epilogue(m, psA_sb[m])

            # ---------------- tail loop over remaining m-tiles ----------------
            # Software-pipelined exactly like the baseline: tile m+2's
            # pack/transpose and tile m-1's epilogue are emitted around tile
            # m's matmuls so the in-order engine queues never stall.
            prev = None
            prev_m = None
            for m in range(A_TILES, M_TILES):
                psum = emit_x_mms(m)
                if m + 2 < M_TILES:
                    emit_x_front(m + 2)
                if prev is not None:
                    emit_epilogue(prev_m, prev)
                prev, prev_m = psum, m
            emit_epilogue(prev_m, prev)


def _build(scale_mul: float, use_b: bool, use_gamma: bool, use_beta: bool):
    """Build the SPMD program (identical on all 8 cores).

    scale_mul = |clip_val| / INTER, folded into the per-channel scale.
    """
    nc = bacc.Bacc("TRN2", target_bir_lowering=False, debug=False,
                   num_devices=N_CORES)

    x_ap = nc.dram_tensor("x", [TPC, INTER], F32, kind="ExternalInput").ap()
    res_ap = nc.dram_tensor("res", [TPC, HIDDEN], F32, kind="ExternalInput").ap()
    wt_ap = nc.dram_tensor("wt", [INTER, HIDDEN], F32, kind="ExternalInput").ap()
    b_ap = g_ap = be_ap = None
    if use_b:
        b_ap = nc.dram_tensor("bvec", [HIDDEN], F32, kind="ExternalInput").ap()
    if use_gamma:
        g_ap = nc.dram_tensor("gvec", [HIDDEN], F32, kind="ExternalInput").ap()
    if use_beta:
        be_ap = nc.dram_tensor("bevec", [HIDDEN], F32, kind="ExternalInput").ap()
    y_ap = nc.dram_tensor("y", [TPC, HIDDEN], F32, kind="ExternalOutput").ap()

    _emit_program(nc, x_ap, res_ap, wt_ap, y_ap, b_ap, g_ap, be_ap,
                  scale_mul, use_b, use_gamma, use_beta)
    nc.compile()
    return nc


_last_results = None


def kernel(hidden_states, input_tensor, W, b, clip_val, gamma, beta):
    global _last_results
    hidden_states = np.asarray(hidden_states)
    input_tensor = np.asarray(input_tensor)
    W = np.asarray(W, dtype=np.float32)
    b = np.asarray(b, dtype=np.float32)
    gamma = np.asarray(gamma, dtype=np.float32)
    beta = np.asarray(beta, dtype=np.float32)
    clip = float(np.asarray(clip_val))

    use_b = bool(np.any(b != 0.0))
    use_gamma = bool(np.any(gamma != 1.0))
    use_beta = bool(np.any(beta != 0.0))
    scale_mul = abs(clip) / INTER

    key = (scale_mul, use_b, use_gamma, use_beta)
    if key not in _cache:
        _cache[key] = _build(scale_mul, use_b, use_gamma, use_beta)
    nc = _cache[key]

    hs = np.ascontiguousarray(
        hidden_states.reshape(TOKENS, INTER).astype(np.float32, copy=False))
    rs = np.ascontiguousarray(
        input_tensor.reshape(TOKENS, HIDDEN).astype(np.float32, copy=False))
    Wc = np.ascontiguousarray(W.T)   # layout choice: weight fed k-major

    in_maps = []
    for c in range(N_CORES):
        m = {
            "x": _prepare_x(hs[c * TPC:(c + 1) * TPC]),
            "res": np.ascontiguousarray(rs[c * TPC:(c + 1) * TPC]),
            "wt": Wc,
        }
        if use_b:
            m["bvec"] = b
        if use_gamma:
            m["gvec"] = gamma
        if use_beta:
            m["bevec"] = beta
        in_maps.append(m)

    kwargs = {}
    if TRACE:
        _install_ntff_hook()
        kwargs["trace"] = True
        if TRACE_ALL_CORES:
            kwargs["trace_cores"] = list(range(N_CORES))
    res = bass_utils.run_bass_kernel_spmd(
        nc, in_maps, core_ids=list(range(N_CORES)), **kwargs)
    _last_results = res

    y = np.concatenate([res.results[c]["y"] for c in range(N_CORES)], axis=0)
    return y.reshape(hidden_states.shape[:-1] + (HIDDEN,)).astype(np.float32)
